# revision 18
# baseline (speedup 1.0000x reference)
"""Trainium2 Bass kernel for nn_MultiHeadAttention_41936060678770.

LinBERT-style linear attention:
  qh/kh/vh = LN(x) @ W + b  (per-stream LN, 16 heads x 64 dim)
  phi = elu(.)+1 ;  phi_k masked
  kv = sum_s phi_k (x) vh ; z = sum_s phi_k
  attn = (phi_q @ kv) / (phi_q @ z + eps)
  out = q + attn @ fc_w + fc_b

Sharding: 8 cores, tokens split 8-ways over flattened (B*S); each pair of
cores (2c, 2c+1) holds one batch, so the [16,64,65] kv/z state is
all-reduced within core pairs; everything else is fully local.

v4 (745us baseline -> 526 -> 485 -> this). Trace-driven changes:
  - host folds LN gain into W, precomputes combined bias, casts weights
    and q/k/v to bf16 (HBM input traffic 42MB -> 21MB);
  - weights load via the otherwise-idle gpsimd queue (one DMA each) so
    the sync queue's ~0.7us/launch budget goes to activation tiles; the
    485us trace lost 53us at startup to weight/load launch contention;
  - all 16 q tiles load + LN + transpose during sweep1's tail (their
    queues have slack), so sweep2's projections start the moment
    sweep1's PE work drains and the kv AllReduce is fully hidden behind
    a 7-tile projection warmup;
  - vh_aug copies and the residual adds run on gpsimd (Pool), exp/xn/
    half the attn scales on ACT, stats/newton/elu-min/stt/den on DVE:
    every engine stays under the PE's ~9.4us/tile;
  - PE p-state: dense back-to-back matmuls ramp the PE to 2.4GHz; idle
    gaps drop it to 1.2GHz, so the whole design minimizes PE stalls.
"""
import sys

sys.path.insert(0, "/opt/trn_rl_repo")

import ml_dtypes
import numpy as np

import concourse.bacc as bacc
import concourse.bass as bass
import concourse.tile as tile
import concourse.mybir as mybir
from concourse.bass_utils import run_bass_kernel_spmd

F32 = mybir.dt.float32
BF16 = mybir.dt.bfloat16
NP_BF16 = ml_dtypes.bfloat16
AF = mybir.ActivationFunctionType
ALU = mybir.AluOpType

B, S, HS = 4, 4096, 1024
NH, D = 16, 64
NCORES = 8
TOK = B * S // NCORES          # 2048 rows per core
NT = TOK // 128                # 16 token tiles
KT = HS // 128                 # 8 hidden tiles
LN_EPS = 1e-5
ATT_EPS = 1e-6
PF = 3                         # sweep1 k/v load prefetch depth (tiles)
WARM = 7                       # sweep2 projection warmup depth (hides CC)


def _ln_stats(nc, stat_pool, x_nat):
    """bn stats + rsig = exp(-0.5*ln(var+eps)) on ACT (verified 1e-5
    accurate on HW). The previous 12-op serial DVE newton chain was a
    pacing hazard: the tile scheduler orders engine streams by dependency
    depth, so the chain's tail sank behind other tiles' stats, delaying
    xn and the PSUM-slot release that projection matmuls wait on.
    Ln/Exp/Identity/Copy share one ACT table set (no reload thrash)."""
    mv = stat_pool.tile([128, 2], F32, tag="mv")
    stats = stat_pool.tile([128, 2, 6], F32, tag="stats")
    nc.vector.bn_stats(out=stats[:, 0, :], in_=x_nat[:, 0:512])
    nc.vector.bn_stats(out=stats[:, 1, :], in_=x_nat[:, 512:1024])
    nc.vector.bn_aggr(out=mv[:], in_=stats[:])
    veps = stat_pool.tile([128, 1], F32, tag="veps")
    nc.vector.tensor_scalar_add(out=veps[:], in0=mv[:, 1:2], scalar1=LN_EPS)
    lnv = stat_pool.tile([128, 1], F32, tag="lnv")
    nc.scalar.activation(out=lnv[:], in_=veps[:], func=AF.Ln)
    sig = stat_pool.tile([128, 1], F32, tag="sig")
    nc.scalar.activation(out=sig[:], in_=lnv[:], func=AF.Exp, scale=-0.5)
    negmusig = stat_pool.tile([128, 1], F32, tag="negmusig")
    nc.vector.scalar_tensor_tensor(
        out=negmusig[:], in0=mv[:, 0:1], scalar=-1.0, in1=sig[:],
        op0=ALU.mult, op1=ALU.mult)
    return sig, negmusig


def _elu1(nc, pool, src_ps, out_ap, mask_col):
    """out = elu(src)+1 = max(src,0) + min(exp(src),1), optionally * mask.
    exp reads PSUM directly on ACT (values here never overflow exp)."""
    texp = pool.tile([128, 512], BF16, tag="texp")
    nc.scalar.activation(out=texp[:], in_=src_ps[:], func=AF.Exp)
    tmin = pool.tile([128, 512], BF16, tag="tmin")
    nc.vector.tensor_scalar_min(out=tmin[:], in0=texp[:], scalar1=1.0)
    if mask_col is None:
        nc.vector.scalar_tensor_tensor(
            out=out_ap, in0=src_ps[:], scalar=0.0, in1=tmin[:],
            op0=ALU.max, op1=ALU.add,
        )
    else:
        tphi = pool.tile([128, 512], F32, tag="tphi")
        nc.vector.scalar_tensor_tensor(
            out=tphi[:], in0=src_ps[:], scalar=0.0, in1=tmin[:],
            op0=ALU.max, op1=ALU.add,
        )
        nc.vector.tensor_scalar_mul(out=out_ap, in0=tphi[:], scalar1=mask_col)


def build(has_c: bool, has_mask: bool, replica_groups,
          _skip_collective=False):
    nc = bacc.Bacc(None)

    qx_d = nc.dram_tensor("qx", [TOK, HS], BF16, kind="ExternalInput")
    kx_d = nc.dram_tensor("kx", [TOK, HS], BF16, kind="ExternalInput")
    vx_d = nc.dram_tensor("vx", [TOK, HS], BF16, kind="ExternalInput")
    w_d = {s: nc.dram_tensor(f"w_{s}", [HS, HS], BF16, kind="ExternalInput")
           for s in ("q", "k", "v", "fc")}
    if has_mask:
        mask_d = nc.dram_tensor("maskx", [TOK, 1], F32, kind="ExternalInput")
    if has_c:
        c_d = {s: nc.dram_tensor(f"c_{s}", [HS], F32, kind="ExternalInput")
               for s in ("q", "k", "v", "fc")}

    out_d = nc.dram_tensor("out", [TOK, HS], F32, kind="ExternalOutput")

    from contextlib import ExitStack
    with tile.TileContext(nc) as tc, ExitStack() as ctx:
        wpool = ctx.enter_context(tc.tile_pool(name="weights", bufs=1))
        consts = ctx.enter_context(tc.tile_pool(name="consts", bufs=1))
        dram_p = ctx.enter_context(
            tc.tile_pool(name="dram", bufs=1, space="DRAM"))
        # q-side pools live across both sweeps (q prep happens in sweep1)
        q_ld = ctx.enter_context(tc.tile_pool(name="q_ld", bufs=NT))
        qw_xn = ctx.enter_context(tc.tile_pool(name="qw_xn", bufs=3))
        qw_xnT = ctx.enter_context(tc.tile_pool(name="qw_xnT", bufs=WARM + 2))
        stat2 = ctx.enter_context(tc.tile_pool(name="stat2", bufs=8))

        # ---------------- weights (bf16 from host, gpsimd queue) ----------
        w_sb = {}
        for s in ("k", "v", "q", "fc"):
            w_sb[s] = wpool.tile([128, KT, HS], BF16, tag=f"w_{s}",
                                 name=f"w_{s}")
            nc.gpsimd.dma_start(
                out=w_sb[s][:],
                in_=w_d[s].rearrange("(kt p) n -> p kt n", p=128))

        c_bc = {"q": None, "k": None, "v": None, "fc": None}
        if has_c:
            for s in ("q", "k", "v", "fc"):
                crow = consts.tile([1, HS], F32, tag=f"crow_{s}")
                nc.sync.dma_start(out=crow[:], in_=c_d[s][None, :])
                c_bc[s] = consts.tile([128, HS], F32, tag=f"cbc_{s}",
                                      name=f"cbc_{s}")
                nc.gpsimd.partition_broadcast(c_bc[s][:], crow[:])

        qloads = {}
        qxnTs = {}

        def emit_load2(i):
            if not (0 <= i < NT):
                return
            r0 = i * 128
            q_nat = q_ld.tile([128, HS], BF16, tag="q_nat")
            nc.sync.dma_start(out=q_nat[:], in_=qx_d[r0:r0 + 128, :])
            qloads[i] = q_nat

        def emit_A2(i):
            """LN + xn + transpose of q tile i (runs during sweep1 tail)."""
            if not (0 <= i < NT):
                return
            q_nat = qloads[i]
            sig, negmusig = _ln_stats(nc, stat2, q_nat)
            xn = qw_xn.tile([128, HS], BF16, tag="xn_q")
            nc.scalar.activation(out=xn[:], in_=q_nat[:], func=AF.Identity,
                                 scale=sig[:], bias=negmusig[:])
            xnT = qw_xnT.tile([128, KT, 128], BF16, tag="xnT_q")
            nc.sync.dma_start_transpose(out=xnT[:], in_=xn[:])
            qxnTs[i] = xnT

        # ---------------- sweep 1: K/V + kv state ----------------
        kv_sb = consts.tile([128, 8, D + 1], F32, tag="kv_sb")
        with (
            tc.tile_pool(name="kv_ps", bufs=1, space="PSUM") as kv_psp,
            tc.tile_pool(name="kh_ps", bufs=3, space="PSUM") as kh_psp,
            tc.tile_pool(name="vh_ps", bufs=3, space="PSUM") as vh_psp,
            tc.tile_pool(name="s1", bufs=3) as s1,
            tc.tile_pool(name="ld1", bufs=6) as ld1,
            tc.tile_pool(name="stat1", bufs=8) as stat1,
        ):
            kv_ps = [kv_psp.tile([128, 4, D + 1], F32, tag=f"kv{b}",
                                 name=f"kv{b}", padded_shape=[128, 4, 128])
                     for b in range(2)]

            loads = {}

            def emit_load1(i):
                if i >= NT:
                    return
                r0 = i * 128
                k_nat = ld1.tile([128, HS], BF16, tag="k_nat")
                nc.sync.dma_start(out=k_nat[:], in_=kx_d[r0:r0 + 128, :])
                v_nat = ld1.tile([128, HS], BF16, tag="v_nat")
                nc.sync.dma_start(out=v_nat[:], in_=vx_d[r0:r0 + 128, :])
                mask_col = None
                if has_mask:
                    mcol = stat1.tile([128, 1], F32, tag="mcol")
                    nc.sync.dma_start(out=mcol[:], in_=mask_d[r0:r0 + 128, :])
                    mask_col = mcol[:]
                loads[i] = (k_nat, v_nat, mask_col)

            def emit_A1(i):
                """LN + xn + transpose for k and v of tile i."""
                if i >= NT:
                    return None
                k_nat, v_nat, mask_col = loads.pop(i)
                res = {}
                for s, x_nat in (("k", k_nat), ("v", v_nat)):
                    sig, negmusig = _ln_stats(nc, stat1, x_nat)
                    xn = s1.tile([128, HS], BF16, tag=f"xn_{s}")
                    nc.scalar.activation(out=xn[:], in_=x_nat[:],
                                         func=AF.Identity,
                                         scale=sig[:], bias=negmusig[:])
                    xnT = s1.tile([128, KT, 128], BF16, tag=f"xnT_{s}")
                    nc.sync.dma_start_transpose(out=xnT[:], in_=xn[:])
                    res[s] = xnT
                res["mask"] = mask_col
                return res

            def emit_B1(i, a):
                """proj k,v + elu(k) + vh_aug for tile i -> (phi_k, vh_aug)."""
                if a is None:
                    return None
                kh_ps = [kh_psp.tile([128, 512], F32, tag="proj",
                                     name="kh_ps")
                         for _ in range(2)]
                for kt in range(KT):
                    for c in range(2):
                        nc.tensor.matmul(
                            kh_ps[c][:], a["k"][:, kt, :],
                            w_sb["k"][:, kt, c * 512:(c + 1) * 512],
                            start=(kt == 0), stop=(kt == KT - 1))
                vh_ps = [vh_psp.tile([128, 512], F32, tag="proj",
                                     name="vh_ps")
                         for _ in range(2)]
                for kt in range(KT):
                    for c in range(2):
                        nc.tensor.matmul(
                            vh_ps[c][:], a["v"][:, kt, :],
                            w_sb["v"][:, kt, c * 512:(c + 1) * 512],
                            start=(kt == 0), stop=(kt == KT - 1))
                phi_k = s1.tile([128, HS], BF16, tag="phi_k")
                for c in range(2):
                    if c_bc["k"] is not None:
                        nc.vector.tensor_tensor(
                            out=kh_ps[c][:], in0=kh_ps[c][:],
                            in1=c_bc["k"][:, c * 512:(c + 1) * 512],
                            op=ALU.add)
                    _elu1(nc, s1, kh_ps[c], phi_k[:, c * 512:(c + 1) * 512],
                          a["mask"])
                vh_aug = s1.tile([128, NH, D + 1], BF16, tag="vh_aug")
                nc.gpsimd.memset(vh_aug[:, :, D:D + 1], 1.0)
                for c in range(2):
                    if c_bc["v"] is not None:
                        nc.vector.tensor_tensor(
                            out=vh_ps[c][:], in0=vh_ps[c][:],
                            in1=c_bc["v"][:, c * 512:(c + 1) * 512],
                            op=ALU.add)
                    nc.scalar.activation(
                        out=vh_aug[:, c * 8:(c + 1) * 8, 0:D],
                        in_=vh_ps[c][:].rearrange("p (n d) -> p n d", d=D),
                        func=AF.Copy)
                return phi_k, vh_aug

            def emit_kv(i, b):
                if b is None:
                    return
                phi_k, vh_aug = b
                for n in range(NH):
                    beta, j, hs = n // 8, (n // 2) % 4, (n % 2) * 64
                    nc.tensor.matmul(
                        kv_ps[beta][hs:hs + 64, j, :],
                        phi_k[:, n * D:(n + 1) * D],
                        vh_aug[:, n, :],
                        start=(i == 0), stop=(i == NT - 1),
                        tile_position=(0, hs),
                        skip_group_check=True,
                    )

            for j in range(PF):
                emit_load1(j)
            a_cur = emit_A1(0)
            b_prev = None
            for i in range(NT):
                emit_load1(i + PF)
                # q prep rides sweep1's spare DVE/ACT/sync capacity:
                # all q loads early, LN+transpose for the first WARM tiles.
                emit_load2(i)
                if i >= NT - WARM:
                    emit_A2(i - (NT - WARM))
                a_next = emit_A1(i + 1)
                b_cur = emit_B1(i, a_cur)
                emit_kv(i - 1, b_prev)
                a_cur, b_prev = a_next, b_cur
            emit_kv(NT - 1, b_prev)

            nc.vector.tensor_copy(out=kv_sb[:, 0:4, :], in_=kv_ps[0][:])
            nc.vector.tensor_copy(out=kv_sb[:, 4:8, :], in_=kv_ps[1][:])

        # ---------------- all-reduce kv state within batch pairs ----------
        # kv2 holds the reduced state as 8 block-diagonal [128, 130] bf16
        # operands (head-pair 2m/2m+1), so the num/den matmul is a plain
        # K=128 matmul at base partition 0.
        kv2 = consts.tile([128, 8, 2 * (D + 1)], BF16, tag="kv2")
        nc.vector.memset(kv2[:], 0.0)
        if _skip_collective:
            nc.vector.tensor_copy(out=kv2[0:64, :, 0:D + 1],
                                  in_=kv_sb[0:64, :, :])
            nc.vector.tensor_copy(out=kv2[64:128, :, D + 1:2 * (D + 1)],
                                  in_=kv_sb[64:128, :, :])
        else:
            cc_in = dram_p.tile([128, 8, D + 1], F32)
            cc_out = dram_p.tile([128, 8, D + 1], F32)
            nc.gpsimd.dma_start(out=cc_in[:], in_=kv_sb[:])
            nc.gpsimd.collective_compute(
                "AllReduce", ALU.add, replica_groups=replica_groups,
                ins=[cc_in.opt()], outs=[cc_out.opt()],
            )
            nc.gpsimd.dma_start(out=kv2[0:64, :, 0:D + 1],
                                in_=cc_out[0:64, :, :])
            nc.gpsimd.dma_start(out=kv2[64:128, :, D + 1:2 * (D + 1)],
                                in_=cc_out[64:128, :, :])

        # ---------------- sweep 2: Q -> attn -> fc -> out ----------------
        with (
            tc.tile_pool(name="proj_ps", bufs=4, space="PSUM") as proj_ps,
            tc.tile_pool(name="nd_ps", bufs=4, space="PSUM") as nd_psp,
            tc.tile_pool(name="s2", bufs=4) as s2,
            tc.tile_pool(name="pqt", bufs=WARM + 1) as pqt,
        ):
            pqts = {}
            attnTs = {}

            def emit_B2(i):
                """q projection + elu + phi_qT for tile i."""
                if not (0 <= i < NT):
                    return
                xnT = qxnTs.pop(i)
                qh_ps = [proj_ps.tile([128, 512], F32, tag="proj",
                                      name="qh_ps")
                         for _ in range(2)]
                for kt in range(KT):
                    for c in range(2):
                        nc.tensor.matmul(
                            qh_ps[c][:], xnT[:, kt, :],
                            w_sb["q"][:, kt, c * 512:(c + 1) * 512],
                            start=(kt == 0), stop=(kt == KT - 1))
                phi_q = s2.tile([128, HS], BF16, tag="phi_q")
                for c in range(2):
                    if c_bc["q"] is not None:
                        nc.vector.tensor_tensor(
                            out=qh_ps[c][:], in0=qh_ps[c][:],
                            in1=c_bc["q"][:, c * 512:(c + 1) * 512],
                            op=ALU.add)
                    _elu1(nc, s2, qh_ps[c], phi_q[:, c * 512:(c + 1) * 512],
                          None)
                phi_qT = pqt.tile([128, KT, 128], BF16, tag="phi_qT")
                nc.sync.dma_start_transpose(out=phi_qT[:], in_=phi_q[:])
                pqts[i] = phi_qT

            def emit_C2(i):
                """nd matmuls + den/rd + attn scaling + attnT for tile i."""
                if not (0 <= i < NT):
                    return
                phi_qT = pqts.pop(i)
                nds = []
                for m in range(8):
                    if m % 2 == 0:
                        nd2 = nd_psp.tile([128, 2, 2 * (D + 1)], F32,
                                          tag="nd", name="nd",
                                          padded_shape=[128, 2, 256])
                        nds.append(nd2)
                    nc.tensor.matmul(
                        nd2[:, m % 2, :], phi_qT[:, m, :], kv2[:, m, :],
                        start=True, stop=True,
                    )
                den = stat2.tile([128, NH], F32, tag="den")
                for p in range(4):
                    nc.vector.tensor_copy(
                        out=den[:, 4 * p:4 * p + 4].rearrange(
                            "a (b c) -> a b c", b=2),
                        in_=nds[p][:, :, D::D + 1])
                rd = stat2.tile([128, NH], F32, tag="rd")
                nc.vector.tensor_scalar_add(out=rd[:], in0=den[:],
                                            scalar1=ATT_EPS)
                nc.vector.reciprocal(out=rd[:], in_=rd[:])
                attn = s2.tile([128, HS], BF16, tag="attn")
                for n in range(NH):
                    nd = nds[n // 4][:, (n // 2) % 2, :]
                    src = nd[:, (n % 2) * (D + 1):(n % 2) * (D + 1) + D]
                    if n % 2 == 0:
                        nc.scalar.activation(
                            out=attn[:, n * D:(n + 1) * D], in_=src,
                            func=AF.Copy, bias=0.0, scale=rd[:, n:n + 1])
                    else:
                        nc.vector.tensor_scalar_mul(
                            out=attn[:, n * D:(n + 1) * D], in0=src,
                            scalar1=rd[:, n:n + 1])
                attnT = s2.tile([128, KT, 128], BF16, tag="attnT")
                nc.sync.dma_start_transpose(out=attnT[:], in_=attn[:])
                attnTs[i] = attnT

            def emit_D2(i):
                """fc + residual + store for tile i."""
                if not (0 <= i < NT):
                    return
                attnT = attnTs.pop(i)
                q_nat = qloads.pop(i)
                fc_ps = [proj_ps.tile([128, 512], F32, tag="proj",
                                      name="fc_ps")
                         for _ in range(2)]
                for kt in range(KT):
                    for c in range(2):
                        nc.tensor.matmul(
                            fc_ps[c][:], attnT[:, kt, :],
                            w_sb["fc"][:, kt, c * 512:(c + 1) * 512],
                            start=(kt == 0), stop=(kt == KT - 1))
                out_sb = s2.tile([128, HS], F32, tag="out_sb")
                for c in range(2):
                    if c_bc["fc"] is not None:
                        nc.vector.tensor_tensor(
                            out=fc_ps[c][:], in0=fc_ps[c][:],
                            in1=c_bc["fc"][:, c * 512:(c + 1) * 512],
                            op=ALU.add)
                    nc.vector.tensor_tensor(
                        out=out_sb[:, c * 512:(c + 1) * 512], in0=fc_ps[c][:],
                        in1=q_nat[:, c * 512:(c + 1) * 512], op=ALU.add)
                r0 = i * 128
                nc.sync.dma_start(out=out_d[r0:r0 + 128, :], in_=out_sb[:])

            for i in range(NT + WARM + 1):
                emit_A2(i + WARM)       # tiles WARM..NT-1 (0..WARM-1 done)
                emit_B2(i)
                emit_C2(i - WARM)
                emit_D2(i - WARM - 1)

    nc.compile()
    return nc


_BUILD_CACHE = {}


def _get_nc(flags, replica_groups):
    key = (flags, tuple(tuple(g) for g in replica_groups))
    if key not in _BUILD_CACHE:
        _BUILD_CACHE[key] = build(*flags, replica_groups)
    return _BUILD_CACHE[key]


def host_prep(q, k, v, ln_q_g, ln_q_b, wq, bq, ln_k_g, ln_k_b, wk, bk,
              ln_v_g, ln_v_b, wv, bv, fc_w, fc_b, mask):
    """Fold LN gains into W, combine biases, cast to bf16. Returns
    (flags, in_maps, groups)."""
    q = np.ascontiguousarray(q, np.float32).reshape(B * S, HS)
    k = np.ascontiguousarray(k, np.float32).reshape(B * S, HS)
    v = np.ascontiguousarray(v, np.float32).reshape(B * S, HS)
    mask_f = np.ascontiguousarray(mask, np.float32).reshape(B * S, 1)

    w_eff = {}
    c_eff = {}
    for s, g, b, w, pb in (("q", ln_q_g, ln_q_b, wq, bq),
                           ("k", ln_k_g, ln_k_b, wk, bk),
                           ("v", ln_v_g, ln_v_b, wv, bv)):
        g = np.asarray(g, np.float32)
        b = np.asarray(b, np.float32)
        w = np.asarray(w, np.float32)
        pb = np.asarray(pb, np.float32)
        we = w * g[:, None] if not np.all(g == 1.0) else w
        w_eff[s] = np.ascontiguousarray(we.astype(NP_BF16))
        c_eff[s] = (b @ we + pb).astype(np.float32)
    w_eff["fc"] = np.ascontiguousarray(
        np.asarray(fc_w, np.float32).astype(NP_BF16))
    c_eff["fc"] = np.asarray(fc_b, np.float32)

    has_c = any(np.any(c != 0.0) for c in c_eff.values())
    has_mask = not bool(np.all(mask_f == 1.0))

    qb = np.ascontiguousarray(q.astype(NP_BF16))
    kb = np.ascontiguousarray(k.astype(NP_BF16))
    vb = np.ascontiguousarray(v.astype(NP_BF16))

    in_maps = []
    for c in range(NCORES):
        r0, r1 = c * TOK, (c + 1) * TOK
        m = {
            "qx": qb[r0:r1], "kx": kb[r0:r1], "vx": vb[r0:r1],
            "w_q": w_eff["q"], "w_k": w_eff["k"], "w_v": w_eff["v"],
            "w_fc": w_eff["fc"],
        }
        if has_mask:
            m["maskx"] = mask_f[r0:r1]
        if has_c:
            for s in ("q", "k", "v", "fc"):
                m[f"c_{s}"] = c_eff[s]
        in_maps.append(m)

    groups = [[0, 1], [2, 3], [4, 5], [6, 7]]
    return (has_c, has_mask), in_maps, groups


def kernel(**inputs):
    flags, in_maps, groups = host_prep(**inputs)
    nc = _get_nc(flags, groups)
    res = run_bass_kernel_spmd(nc, in_maps, list(range(NCORES)))
    out = np.concatenate([res.results[c]["out"] for c in range(NCORES)], 0)
    return out.reshape(B, S, HS).astype(np.float32)
